# revision 14
# baseline (speedup 1.0000x reference)
"""Multi-head attention Trainium2 kernel (8 NeuronCores, head-parallel, fp8).

Reference computation (B=4, S=1024, D=512, H=8, per-head dim == D):
    Q = (query @ Wq) -> [B,H,S,D];  K, V likewise
    scores = Q K^T / sqrt(D), masked (mask==0 -> -1e6), softmax over keys
    ctx = attn @ V;  out = query + concat(ctx) @ Wo + bo

Sharding: one head per core (tensor parallel).  Each core computes its
head's partial output  ctx_h @ Wo_h; the host sums the 8 partials (the
all-reduce), adds the residual + bias, and reshapes.

Device-side strategy (per core):
  - all matmuls run in fp8 (float8e4, TRN max 240) with
    MatmulPerfMode.DoubleRow: each instruction contracts 2x128=256 via
    operand pairs shaped [128, 2, *], ~1.4-1.8x the bf16 PE rate.
  - scaling keeps every fp8 tensor in the e4m3 sweet spot: weights are
    shipped x16 (Wq/Wk/Wv) and x64 (Wo); Q/K PSUM->SBUF copies divide
    by 16 (so scores PSUM is true-scale); V stays x16 (ctx ~O(1));
    output partials leave the chip x1024 in bf16 and the host divides.
  - softmax: exp(scale*scores - 2) via ACT (bias kills any chance of
    fp8 overflow; the -2 cancels in normalization), mask applied
    multiplicatively on GpSimd (SBUF-only engine), denominators via a
    ones-lhsT DoubleRow matmul -> fast approximate reciprocal; the
    divide rides the ctx PSUM->SBUF copy as a tensor_tensor multiply.
  - post-matmul copies are split between ACT (K/V copies) and DVE
    (Q copies, ctx normalize, out copies) so no engine approaches the
    PE's busy time.
"""

import sys

if "/opt/trn_rl_repo" not in sys.path:
    sys.path.insert(0, "/opt/trn_rl_repo")

import numpy as np

B, S, D, H = 4, 1024, 512, 8
N_CORES = 8
P = 128
DC = D // P           # d_model chunks          (4)
JC = D // P           # head-dim chunks         (4)
KC = S // P           # key chunks per batch    (8)
NQ = 512              # q-tile size (half of a batch's sequence)
QH = S // NQ          # q-tiles per batch       (2)
NCOL = S // NQ        # n-column tiles for K/V projections (2)
SCALE = 1.0 / float(np.sqrt(D))
EXP_BIAS = -2.0       # cancels in softmax; keeps exp outputs < 40
WSCALE_QKV = 16.0     # host premultiplies Wq/Wk/Wv (fp8 sweet spot)
WSCALE_O = 64.0       # host premultiplies Wo
OUT_DESCALE = 1.0 / (WSCALE_QKV * WSCALE_O)   # host-side 1/1024

_PROG = None          # cached compiled Bass module
LAST_RESULTS = None   # results of the last run (for test harness)


def _build_program():
    import concourse.bacc as bacc
    import concourse.tile as tile
    import concourse.mybir as mybir
    from contextlib import ExitStack

    f32 = mybir.dt.float32
    bf16 = mybir.dt.bfloat16
    fp8 = mybir.dt.float8e4
    EXP = mybir.ActivationFunctionType.Exp
    MUL = mybir.AluOpType.mult
    DR = mybir.MatmulPerfMode.DoubleRow

    nc = bacc.Bacc("TRN2", target_bir_lowering=False, debug=False,
                   num_devices=N_CORES)

    # wire layouts are chosen so every DMA is one fully-contiguous dram
    # block per partition run (partition stride == per-partition bytes):
    # strided patterns collapse HBM efficiency ~8x.
    qt = nc.dram_tensor("qt", [B, P, DC, S], fp8, kind="ExternalInput").ap()
    kt = nc.dram_tensor("kt", [B, P, DC, S], fp8, kind="ExternalInput").ap()
    vt = nc.dram_tensor("vt", [B, P, DC, S], fp8, kind="ExternalInput").ap()
    mkt = nc.dram_tensor("maskt", [B, P, KC, S], fp8,
                         kind="ExternalInput").ap()
    wq = nc.dram_tensor("wq", [P, DC, JC, P], fp8, kind="ExternalInput").ap()
    wk = nc.dram_tensor("wk", [P, DC, JC, P], fp8, kind="ExternalInput").ap()
    wv = nc.dram_tensor("wv", [P, DC, D], fp8, kind="ExternalInput").ap()
    wo = nc.dram_tensor("wo", [P, JC, DC, P], fp8, kind="ExternalInput").ap()
    outt = nc.dram_tensor("outt", [B, QH, P, DC, NQ], bf16,
                          kind="ExternalOutput").ap()

    with tile.TileContext(nc) as tc, ExitStack() as ctx:
        wp = ctx.enter_context(tc.tile_pool(name="wp", bufs=1))
        kin_p = ctx.enter_context(tc.tile_pool(name="kin_p", bufs=2))
        vin_p = ctx.enter_context(tc.tile_pool(name="vin_p", bufs=2))
        qin_p = ctx.enter_context(tc.tile_pool(name="qin_p", bufs=2))
        kv_p = ctx.enter_context(tc.tile_pool(name="kv_p", bufs=2))
        qtp = ctx.enter_context(tc.tile_pool(name="qtp", bufs=2))
        ex_p = ctx.enter_context(tc.tile_pool(name="ex_p", bufs=2))
        mk_p = ctx.enter_context(tc.tile_pool(name="mk_p", bufs=2))
        cx_p = ctx.enter_context(tc.tile_pool(name="cx_p", bufs=2))
        ot_p = ctx.enter_context(tc.tile_pool(name="ot_p", bufs=2))
        rb_p = ctx.enter_context(tc.tile_pool(name="rb_p", bufs=2))
        ef_p = ctx.enter_context(tc.tile_pool(name="ef_p", bufs=3))
        psA = ctx.enter_context(tc.tile_pool(name="psA", bufs=2, space="PSUM"))
        psS = ctx.enter_context(tc.tile_pool(name="psS", bufs=2, space="PSUM"))
        psC = ctx.enter_context(tc.tile_pool(name="psC", bufs=2, space="PSUM"))
        psM = ctx.enter_context(tc.tile_pool(name="psM", bufs=2, space="PSUM"))

        # ---- persistent weights / constants ----
        wq_sb = wp.tile([P, DC, JC, P], fp8)
        wk_sb = wp.tile([P, DC, JC, P], fp8)
        wv_sb = wp.tile([P, DC, D], fp8)
        wo_sb = wp.tile([P, JC, DC, P], fp8)
        ones_mat = wp.tile([P, 2, P], fp8)
        bias_t = wp.tile([P, 1], f32)

        # batch-sized input tiles (1KB partition lines).  DMA issues cost
        # ~1.2us on the issuing queue, so the first batch's loads are
        # spread across queues to start compute ASAP; steady-state input
        # prefetch rides the sync queue (a full batch of slack).
        def dma_kin(b, eng=None):
            t = kin_p.tile([P, DC, S], fp8, tag="kin", name="kin_t")
            (eng or nc.sync).dma_start(t[:], kt[b])
            return t

        def dma_vin(b, eng=None):
            t = vin_p.tile([P, DC, S], fp8, tag="vin", name="vin_t")
            (eng or nc.sync).dma_start(t[:], vt[b])
            return t

        def dma_qin(b, eng=None):
            t = qin_p.tile([P, DC, S], fp8, tag="qin", name="qin_t")
            (eng or nc.sync).dma_start(t[:], qt[b])
            return t

        def dma_mask(b, eng=None, split=False):
            t = mk_p.tile([P, KC, S], fp8, tag="mk", name="mk_t")
            if split:
                nc.gpsimd.dma_start(t[:, :, 0:NQ], mkt[b][:, :, 0:NQ])
                nc.gpsimd.dma_start(t[:, :, NQ:S], mkt[b][:, :, NQ:S])
            else:
                (eng or nc.gpsimd).dma_start(t[:], mkt[b])
            return t

        nc.scalar.dma_start(wk_sb[:], wk[:])
        kin_t = dma_kin(0)
        vin_t = dma_vin(0, eng=nc.scalar)
        nc.scalar.dma_start(wv_sb[:], wv[:])
        qin_t = dma_qin(0, eng=nc.sync)
        mk_t = dma_mask(0, split=True)
        nc.scalar.dma_start(wq_sb[:], wq[:])
        nc.gpsimd.dma_start(wo_sb[:], wo[:])
        nc.vector.memset(ones_mat[:], 1.0)
        nc.vector.memset(bias_t[:], EXP_BIAS)

        def emit_qproj(qin_t, qh):
            """Q projection for one q-tile from the batch qin tile."""
            QT = qtp.tile([P, JC, NQ], fp8, tag="QT", name="QT")
            for jc in range(JC):
                pp = psA.tile([P, NQ], f32, tag="pproj", name="pp")
                for dcp in range(DC // 2):
                    nc.tensor.matmul(pp[:], wq_sb[:, 2 * dcp:2 * dcp + 2, jc, :],
                                     qin_t[:, 2 * dcp:2 * dcp + 2,
                                           qh * NQ:(qh + 1) * NQ],
                                     start=(dcp == 0), stop=(dcp == DC // 2 - 1),
                                     perf_mode=DR)
                # QT stores Q/16 so the scores PSUM comes out true-scale
                nc.vector.tensor_scalar_mul(QT[:, jc, :], pp[:],
                                            1.0 / WSCALE_QKV)
            return QT

        nxt = None
        for b in range(B):
            base = b * S
            # ---- K/V projections for the whole batch ----
            KT = kv_p.tile([P, JC, S], fp8, tag="KT")    # [j, n] keys^T / 16
            V = kv_p.tile([P, KC, D], fp8, tag="V")      # [n, dv] values x16
            for half in range(NCOL):
                for jc in range(JC):
                    pp = psA.tile([P, NQ], f32, tag="pproj")
                    for dcp in range(DC // 2):
                        nc.tensor.matmul(
                            pp[:], wk_sb[:, 2 * dcp:2 * dcp + 2, jc, :],
                            kin_t[:, 2 * dcp:2 * dcp + 2,
                                  half * NQ:(half + 1) * NQ],
                            start=(dcp == 0), stop=(dcp == DC // 2 - 1),
                            perf_mode=DR)
                    nc.scalar.mul(KT[:, jc, half * NQ:(half + 1) * NQ],
                                  pp[:], 1.0 / WSCALE_QKV)
                for k2 in range(NQ // P):
                    kc = half * (NQ // P) + k2
                    pp = psA.tile([P, D], f32, tag="pproj")
                    for dcp in range(DC // 2):
                        nc.tensor.matmul(
                            pp[:], vin_t[:, 2 * dcp:2 * dcp + 2,
                                         kc * P:(kc + 1) * P],
                            wv_sb[:, 2 * dcp:2 * dcp + 2, :],
                            start=(dcp == 0), stop=(dcp == DC // 2 - 1),
                            perf_mode=DR)
                    nc.scalar.copy(V[:, kc, :], pp[:])

            # prefetch next batch's inputs; current tiles stay live
            cur_qin, cur_mk = qin_t, mk_t
            if b + 1 < B:
                kin_t = dma_kin(b + 1)
                vin_t = dma_vin(b + 1)
                qin_t = dma_qin(b + 1)
                mk_t = dma_mask(b + 1)

            if b == 0:
                nxt = emit_qproj(cur_qin, 0)

            for qh in range(QH):
                col = base + qh * NQ
                QT = nxt

                # ---- scores^T (true scale), exp(x/sqrt(D)-2), mask ----
                ex_t = ex_p.tile([P, KC, NQ], fp8, tag="ex")
                for kc in range(KC):
                    ps = psS.tile([P, NQ], f32, tag="pscore")
                    for jcp in range(JC // 2):
                        nc.tensor.matmul(
                            ps[:],
                            KT[:, 2 * jcp:2 * jcp + 2, kc * P:(kc + 1) * P],
                            QT[:, 2 * jcp:2 * jcp + 2, :],
                            start=(jcp == 0), stop=(jcp == JC // 2 - 1),
                            perf_mode=DR)
                    # fp8 ef is free accuracy-wise: mask is 0/1 so the
                    # post-mask quantization is identical
                    ef_t = ef_p.tile([P, NQ], fp8, tag="expf")
                    nc.scalar.activation(ef_t[:], ps[:], EXP,
                                         bias=bias_t[:], scale=SCALE)
                    nc.gpsimd.tensor_tensor(
                        ex_t[:, kc, :], ef_t[:],
                        cur_mk[:, kc, qh * NQ:(qh + 1) * NQ], MUL)

                # ---- next q-tile's projection fills the PE while the
                #      exp/mask chain drains ----
                if qh + 1 < QH:
                    nxt = emit_qproj(cur_qin, qh + 1)
                elif b + 1 < B:
                    nxt = emit_qproj(qin_t, 0)

                # ---- softmax denominator, replicated across partitions:
                #      ones[128,2,128]^T @ ex gives sum_k on every partition
                pr = psM.tile([P, NQ], f32, tag="pmix")
                for kcp in range(KC // 2):
                    nc.tensor.matmul(pr[:], ones_mat[:],
                                     ex_t[:, 2 * kcp:2 * kcp + 2, :],
                                     start=(kcp == 0), stop=(kcp == KC // 2 - 1),
                                     perf_mode=DR)
                rb = rb_p.tile([P, NQ], f32, tag="rb")
                nc.vector.reciprocal_approx_fast(rb[:], pr[:])

                # ---- ctx^T = V^T @ attn (x16), normalized ----
                ctx_t = cx_p.tile([P, JC, NQ], fp8, tag="ctx")
                for dvc in range(JC):
                    pc = psC.tile([P, NQ], f32, tag="pctx")
                    for kcp in range(KC // 2):
                        nc.tensor.matmul(
                            pc[:],
                            V[:, 2 * kcp:2 * kcp + 2, dvc * P:(dvc + 1) * P],
                            ex_t[:, 2 * kcp:2 * kcp + 2, :],
                            start=(kcp == 0), stop=(kcp == KC // 2 - 1),
                            perf_mode=DR)
                    nc.vector.tensor_tensor(ctx_t[:, dvc, :], pc[:], rb[:], MUL)

                # ---- out^T partial = Wo_h^T ctx^T  (x1024, bf16 wire) ----
                ot_t = ot_p.tile([P, DC, NQ], bf16, tag="ot")
                for oc in range(DC):
                    po = psM.tile([P, NQ], f32, tag="pmix")
                    for dvp in range(JC // 2):
                        nc.tensor.matmul(po[:],
                                         wo_sb[:, 2 * dvp:2 * dvp + 2, oc, :],
                                         ctx_t[:, 2 * dvp:2 * dvp + 2, :],
                                         start=(dvp == 0),
                                         stop=(dvp == JC // 2 - 1),
                                         perf_mode=DR)
                    nc.vector.tensor_copy(ot_t[:, oc, :], po[:])
                # one contiguous 256KB store per q-tile
                nc.sync.dma_start(outt[b, qh], ot_t[:])

    nc.compile()
    return nc


def _get_program():
    global _PROG
    if _PROG is None:
        _PROG = _build_program()
    return _PROG


def _fp8(x):
    import ml_dtypes
    return np.clip(x, -240.0, 240.0).astype(ml_dtypes.float8_e4m3)


def _lhsT_layout(w):          # [D, D] -> [P, DC, JC, P]
    return np.ascontiguousarray(w.reshape(DC, P, JC, P).transpose(1, 0, 2, 3))


def _rhs_layout(w):           # [D, D] -> [P, DC, D]
    return np.ascontiguousarray(w.reshape(DC, P, D).transpose(1, 0, 2))


def _in_layout(x):            # [B,S,D] -> [B, P, DC, S] (contiguous DMA)
    return np.ascontiguousarray(
        np.asarray(x, dtype=np.float32).reshape(B, S, DC, P)
        .transpose(0, 3, 2, 1))


def prepare_in_maps(query, key, value, mask, Wq, Wk, Wv, Wo):
    qt = _fp8(_in_layout(query))
    kt = _fp8(_in_layout(key))
    vt = _fp8(_in_layout(value))
    # mask^T [b,k,q] -> [B, P, KC, S]
    mk = _fp8(np.ascontiguousarray(
        np.asarray(mask).transpose(0, 2, 1).astype(np.float32)
        .reshape(B, KC, P, S).transpose(0, 2, 1, 3)))
    Wq = np.asarray(Wq, dtype=np.float32) * WSCALE_QKV
    Wk = np.asarray(Wk, dtype=np.float32) * WSCALE_QKV
    Wv = np.asarray(Wv, dtype=np.float32) * WSCALE_QKV
    Wo = np.asarray(Wo, dtype=np.float32) * WSCALE_O

    in_maps = []
    for h in range(N_CORES):
        sl = slice(h * D, (h + 1) * D)
        in_maps.append({
            "qt": qt, "kt": kt, "vt": vt, "maskt": mk,
            "wq": _fp8(_lhsT_layout(Wq[:, sl])),
            "wk": _fp8(_lhsT_layout(Wk[:, sl])),
            "wv": _fp8(_rhs_layout(Wv[:, sl])),
            "wo": _fp8(_lhsT_layout(Wo[sl, :])),
        })
    return in_maps


def postprocess(results, query, bo):
    acc = results[0]["outt"].astype(np.float32)
    for c in range(1, N_CORES):
        acc += results[c]["outt"]
    # [B, QH, P, DC, NQ] -> [B, S, D]
    out = np.ascontiguousarray(
        acc.transpose(0, 1, 4, 3, 2)).reshape(B, S, D) * OUT_DESCALE
    out += np.asarray(query, dtype=np.float32)
    out += np.asarray(bo, dtype=np.float32)[None, None, :]
    return out


def kernel(query, key, value, mask, Wq, Wk, Wv, Wo, bo):
    global LAST_RESULTS
    from concourse.bass_utils import run_bass_kernel_spmd

    nc = _get_program()
    in_maps = prepare_in_maps(query, key, value, mask, Wq, Wk, Wv, Wo)
    res = run_bass_kernel_spmd(nc, in_maps, list(range(N_CORES)))
    LAST_RESULTS = res
    return postprocess(res.results, query, bo)
